# revision 10
# baseline (speedup 1.0000x reference)
"""GateRow kernel for Trainium2 (8 NeuronCores, SPMD, gate-sharded, bit-packed).

Problem: out[b, g] = gates[g, 2*x[b, c0[g]] + x[b, c1[g]]]
  x: [16384, 8192] bool, gates: [8192, 4] bool, choices: [8192, 2] int32.

Strategy:
  Every 2-input boolean gate is  rowA OP rowB  for OP in {AND, OR, XOR}
  once operand inversion and constants are absorbed into a doubled
  lookup table TAB = [x^T ; ~x^T ; ones ; zeros] (one row per wire).
  Bit-pack the batch dimension (8 rows/byte) so each TAB row is
  B/8 = 2048 bytes and the boolean op is a plain bitwise op (done on
  uint32 views: bitwise is byte-local, and 32-bit elements quarter the
  DVE element count).

  Shard by GATES: core k owns 1024 gates.  Host sorts gates into
  type-homogeneous blocks of 128 under a fixed per-core schedule
  (3 AND blocks, 3 OR blocks, 2 XOR blocks); "flexible" gates
  (constants / projections, expressible in any family) pad the
  buckets to exact capacity.  The host un-permutes output columns.

  Device (per core): dma_gathers (2048 rows, 2048 B/row, 4 MB total),
  8 stock tensor_tensor bitwise ops, 8 output DMAs (2 MB total).
  No PE, no PSUM, no custom DVE ops.
"""

import sys

for _p in ("/opt/trn_rl_repo", "/opt/pypackages"):
    if _p not in sys.path:
        sys.path.append(_p)

from contextlib import ExitStack

import numpy as np

import concourse.bass as bass
import concourse.bacc as bacc
import concourse.tile as tile
import concourse.mybir as mybir
from concourse.bass_utils import run_bass_kernel_spmd

B, N, G, NCORES = 16384, 8192, 8192, 8
GPC = G // NCORES           # 1024 gates per core
NBLK = GPC // 128           # 8 gate blocks per core
PB = B // 8                 # 2048 packed bytes per table row
PW = PB // 4                # 512 packed uint32 words per table row
ROW_ONE = 2 * N             # all-ones table row
ROW_ZERO = 2 * N + 1        # all-zeros table row
NCALLS = 4                  # dma_gather calls (a+b interleaved per call)

# Per-core block op schedule: 3 AND, 3 OR, 2 XOR blocks of 128 gates.
SCHED = ("and",) * 3 + ("or",) * 3 + ("xor",) * 2
CAP = {"and": 3 * 128 * NCORES, "or": 3 * 128 * NCORES, "xor": 2 * 128 * NCORES}

# ---------------------------------------------------------------------------
# Gate classification.
#   tt bit (2a+b) = f(a, b).  Operand selectors:
#     0: x[c0]   1: ~x[c0]   2: x[c1]   3: ~x[c1]   4: ones   5: zeros
#   SEL[op][tt] = (selA, selB) with f == rowA op rowB; None if inexpressible.
# ---------------------------------------------------------------------------

_OPS = ("and", "or", "xor")
_NPOP = {"and": np.bitwise_and, "or": np.bitwise_or, "xor": np.bitwise_xor}


def _build_sel():
    sel = {op: [None] * 16 for op in _OPS}
    for tt in range(16):
        for op in _OPS:
            for sa in range(6):
                for sb in range(6):
                    ok = True
                    for a in (0, 1):
                        for b in (0, 1):
                            va = (a, 1 - a, b, 1 - b, 1, 0)[sa]
                            vb = (a, 1 - a, b, 1 - b, 1, 0)[sb]
                            r = int(_NPOP[op](va, vb))
                            if r != ((tt >> (2 * a + b)) & 1):
                                ok = False
                    if ok and sel[op][tt] is None:
                        sel[op][tt] = (sa, sb)
    return sel


_SEL = _build_sel()
_FAMS = [frozenset(op for op in _OPS if _SEL[op][tt] is not None) for tt in range(16)]


# ---------------------------------------------------------------------------
# Device program
# ---------------------------------------------------------------------------

_ALU = {
    "and": mybir.AluOpType.bitwise_and,
    "or": mybir.AluOpType.bitwise_or,
    "xor": mybir.AluOpType.bitwise_xor,
}


def build_nc(ncalls=NCALLS, ncores=NCORES):
    """One SPMD program; all cores run it on their own gate shard.

    ncalls dma_gather calls; each gathers the A then B rows for
    NBLK/ncalls consecutive gate blocks (interleaved a,b per call group
    so compute on group i overlaps the gather of group i+1).
    """
    npc = NBLK // ncalls     # gate blocks per call group
    nidx = npc * 2 * 128     # rows per dma_gather call (a rows then b rows)
    percall = nidx // 16     # int16s per partition per call

    nc = bacc.Bacc(
        "TRN2",
        target_bir_lowering=False,
        debug=False,
        num_devices=ncores,
        num_swdge_queues=4,
    )
    tab = nc.dram_tensor("tab", [2 * N + 2, PW], mybir.dt.uint32, kind="ExternalInput")
    idxs = nc.dram_tensor(
        "idxs", [128, ncalls * percall], mybir.dt.int16, kind="ExternalInput"
    )
    outd = nc.dram_tensor("out", [GPC, PW], mybir.dt.uint32, kind="ExternalOutput")

    with tile.TileContext(nc) as tc, ExitStack() as ctx:
        pconst = ctx.enter_context(tc.tile_pool(name="const", bufs=1))
        pg = ctx.enter_context(tc.tile_pool(name="gather", bufs=2))
        po = ctx.enter_context(tc.tile_pool(name="osb", bufs=2))

        # Load indices with the gpsimd engine's own DMA so the gather's
        # dependency is engine-local (no slow cross-engine sem wait).
        idx_t = pconst.tile([128, idxs.shape[1]], mybir.dt.int16)
        nc.gpsimd.dma_start(idx_t[:], idxs[:])

        # Warm-up: a tiny gather whose indices come from an on-chip memset
        # (no DMA dependency), so the one-time SWDGE ring/ucode init runs
        # while the index load is still in flight.
        widx_t = pconst.tile([128, 8], mybir.dt.int16)
        nc.gpsimd.memset(widx_t[:], 0)
        w_t = pconst.tile([128, 1, PW], mybir.dt.uint32)
        nc.gpsimd.dma_gather(
            w_t[:], tab[:], widx_t[:], 128, 128, PW, single_packet=False
        )

        for h in range(ncalls):
            g_t = pg.tile([128, 2 * npc, PW], mybir.dt.uint32, tag="g")
            nc.gpsimd.dma_gather(
                g_t[:],
                tab[:],
                idx_t[:, h * percall : (h + 1) * percall],
                nidx,
                nidx,
                PW,
                single_packet=False,
                queue_num=h % 4,
            )
            for j in range(npc):
                bk = h * npc + j
                o_t = po.tile([128, PW], mybir.dt.uint32, tag=f"o{bk}")
                nc.vector.tensor_tensor(
                    o_t[:],
                    g_t[:, 2 * j, :],
                    g_t[:, 2 * j + 1, :],
                    op=_ALU[SCHED[bk]],
                )
                nc.sync.dma_start(outd[bk * 128 : (bk + 1) * 128, :], o_t[:])
    nc.compile()
    return nc


# ---------------------------------------------------------------------------
# Host-side input prep
# ---------------------------------------------------------------------------


def _prep(x, gates, choices, ncalls=NCALLS):
    x8 = np.asarray(x, dtype=np.uint8)
    gates8 = np.asarray(gates, dtype=np.uint8)
    ch = np.asarray(choices, dtype=np.int64)

    # Packed doubled table (replicated on every core).
    xp = np.packbits(x8, axis=0)              # [B/8, N], bit MSB = lowest batch row
    tab = np.empty((2 * N + 2, PB), dtype=np.uint8)
    tab[:N] = xp.T
    tab[N : 2 * N] = ~tab[:N]
    tab[ROW_ONE] = 0xFF
    tab[ROW_ZERO] = 0x00
    tab32 = tab.view(np.uint32)

    # Bucket assignment: required-family gates first, flexible gates pad.
    tt = (gates8 << np.arange(4, dtype=np.uint8)).sum(axis=1).astype(np.int64)
    req = {op: [t for t in range(16) if _FAMS[t] == {op}] for op in _OPS}
    flex = [t for t in range(16) if len(_FAMS[t]) == 3]
    assert sum(len(v) for v in req.values()) + len(flex) == 16

    gid = np.arange(G)
    flex_pool = gid[np.isin(tt, flex)]
    fp = 0
    slots = {}
    for op in _OPS:
        need = gid[np.isin(tt, req[op])]
        pad = CAP[op] - len(need)
        assert pad >= 0, f"bucket {op} overflow: {len(need)} > {CAP[op]}"
        slots[op] = np.concatenate([need, flex_pool[fp : fp + pad]])
        fp += pad
    assert fp == len(flex_pool)

    # Device gate order (core-major, schedule-major) + operand row indices.
    npcg = {"and": 3 * 128, "or": 3 * 128, "xor": 2 * 128}
    perm = np.empty(G, dtype=np.int64)        # device row -> gate id
    ia = np.empty(G, dtype=np.int64)
    ib = np.empty(G, dtype=np.int64)
    r = 0
    for k in range(NCORES):
        for op in _OPS:
            g = slots[op][k * npcg[op] : (k + 1) * npcg[op]]
            lut = [_SEL[op][t] or (5, 5) for t in range(16)]  # (5,5) never used
            selA = np.array([s[0] for s in lut])[tt[g]]
            selB = np.array([s[1] for s in lut])[tt[g]]
            rows = np.stack(
                [ch[g, 0], ch[g, 0] + N, ch[g, 1], ch[g, 1] + N,
                 np.full(len(g), ROW_ONE), np.full(len(g), ROW_ZERO)]
            )
            n = len(g)
            perm[r : r + n] = g
            ia[r : r + n] = rows[selA, np.arange(n)]
            ib[r : r + n] = rows[selB, np.arange(n)]
            r += n
    assert r == G

    # Wrapped int16 index layout per core: ncalls calls; call h covers
    # npc gate blocks -> flat order [a rows of npc blocks, b rows of npc
    # blocks] interleaved as [a(blk0),b(blk0),a(blk1),b(blk1),...] to
    # match tile slots (2j, 2j+1).
    npc = NBLK // ncalls
    in_maps = []
    for k in range(NCORES):
        s = slice(k * GPC, (k + 1) * GPC)
        iak = ia[s].reshape(NBLK, 128)
        ibk = ib[s].reshape(NBLK, 128)
        cols = []
        for h in range(ncalls):
            inter = np.empty((2 * npc, 128), dtype=np.int16)
            inter[0::2] = iak[h * npc : (h + 1) * npc]
            inter[1::2] = ibk[h * npc : (h + 1) * npc]
            flat = inter.reshape(-1)
            wrapped = flat.reshape(-1, 16).T      # [16, nidx/16]
            cols.append(np.tile(wrapped, (8, 1)))  # [128, nidx/16]
        idxs_np = np.ascontiguousarray(np.concatenate(cols, axis=1))
        in_maps.append({"tab": tab32, "idxs": idxs_np})
    return in_maps, perm


# ---------------------------------------------------------------------------
# Entry point
# ---------------------------------------------------------------------------

_NC_CACHE = {}


def _get_nc():
    if "nc" not in _NC_CACHE:
        _NC_CACHE["nc"] = build_nc()
    return _NC_CACHE["nc"]


def kernel(x, gates, choices):
    in_maps, perm = _prep(x, gates, choices)
    nc = _get_nc()
    res = run_bass_kernel_spmd(nc, in_maps, list(range(NCORES)))
    packed = np.concatenate(
        [res.results[k]["out"].view(np.uint8) for k in range(NCORES)], axis=0
    )
    ordered = np.empty_like(packed)
    ordered[perm] = packed                    # un-permute gate rows
    up = np.unpackbits(ordered, axis=1)       # [G, B] 0/1 uint8
    return up.view(np.bool_).T                # [B, G] bool view


# revision 13
# speedup vs baseline: 1.1577x; 1.1577x over previous
"""GateRow kernel for Trainium2 (8 NeuronCores, SPMD, gate-sharded, bit-packed).

Problem: out[b, g] = gates[g, 2*x[b, c0[g]] + x[b, c1[g]]]
  x: [16384, 8192] bool, gates: [8192, 4] bool, choices: [8192, 2] int32.

Strategy:
  Every 2-input boolean gate is  rowA OP rowB  for OP in {AND, OR, XOR}
  once operand inversion and constants are absorbed into a doubled
  lookup table TAB = [x^T ; ~x^T ; ones ; zeros] (one row per wire).
  Bit-pack the batch dimension (8 rows/byte) so each TAB row is
  B/8 = 2048 bytes and the boolean op is a plain bitwise op (done on
  uint32 views: bitwise is byte-local, and 32-bit elements quarter the
  DVE element count).

  Shard by GATES: core k owns 1024 gates.  Host sorts gates into
  type-homogeneous blocks of 128 under a fixed per-core schedule
  (3 AND blocks, 3 OR blocks, 2 XOR blocks); "flexible" gates
  (constants / projections, expressible in any family) pad the
  buckets to exact capacity.  The host un-permutes output columns.

  Device (per core): dma_gathers (2048 rows, 2048 B/row, 4 MB total),
  8 stock tensor_tensor bitwise ops, 8 output DMAs (2 MB total).
  No PE, no PSUM, no custom DVE ops.
"""

import sys

for _p in ("/opt/trn_rl_repo", "/opt/pypackages"):
    if _p not in sys.path:
        sys.path.append(_p)

from contextlib import ExitStack

import numpy as np

import concourse.bass as bass
import concourse.bacc as bacc
import concourse.tile as tile
import concourse.mybir as mybir
from concourse.bass_utils import run_bass_kernel_spmd

B, N, G, NCORES = 16384, 8192, 8192, 8
GPC = G // NCORES           # 1024 gates per core
NBLK = GPC // 128           # 8 gate blocks per core
PB = B // 8                 # 2048 packed bytes per table row
PW = PB // 4                # 512 packed uint32 words per table row
ROW_ONE = 2 * N             # all-ones table row
ROW_ZERO = 2 * N + 1        # all-zeros table row
NCALLS = 4                  # dma_gather calls (a+b interleaved per call)

# Per-core block op schedule: 3 AND, 3 OR, 2 XOR blocks of 128 gates.
SCHED = ("and",) * 3 + ("or",) * 3 + ("xor",) * 2
CAP = {"and": 3 * 128 * NCORES, "or": 3 * 128 * NCORES, "xor": 2 * 128 * NCORES}

# ---------------------------------------------------------------------------
# Gate classification.
#   tt bit (2a+b) = f(a, b).  Operand selectors:
#     0: x[c0]   1: ~x[c0]   2: x[c1]   3: ~x[c1]   4: ones   5: zeros
#   SEL[op][tt] = (selA, selB) with f == rowA op rowB; None if inexpressible.
# ---------------------------------------------------------------------------

_OPS = ("and", "or", "xor")
_NPOP = {"and": np.bitwise_and, "or": np.bitwise_or, "xor": np.bitwise_xor}


def _build_sel():
    sel = {op: [None] * 16 for op in _OPS}
    for tt in range(16):
        for op in _OPS:
            for sa in range(6):
                for sb in range(6):
                    ok = True
                    for a in (0, 1):
                        for b in (0, 1):
                            va = (a, 1 - a, b, 1 - b, 1, 0)[sa]
                            vb = (a, 1 - a, b, 1 - b, 1, 0)[sb]
                            r = int(_NPOP[op](va, vb))
                            if r != ((tt >> (2 * a + b)) & 1):
                                ok = False
                    if ok and sel[op][tt] is None:
                        sel[op][tt] = (sa, sb)
    return sel


_SEL = _build_sel()
_FAMS = [frozenset(op for op in _OPS if _SEL[op][tt] is not None) for tt in range(16)]


# ---------------------------------------------------------------------------
# Device program
# ---------------------------------------------------------------------------

_ALU = {
    "and": mybir.AluOpType.bitwise_and,
    "or": mybir.AluOpType.bitwise_or,
    "xor": mybir.AluOpType.bitwise_xor,
}


def build_nc(ncalls=NCALLS, ncores=NCORES):
    """One SPMD program; all cores run it on their own gate shard.

    ncalls dma_gather calls; each gathers the A then B rows for
    NBLK/ncalls consecutive gate blocks (interleaved a,b per call group
    so compute on group i overlaps the gather of group i+1).
    """
    npc = NBLK // ncalls     # gate blocks per call group
    nidx = npc * 2 * 128     # rows per dma_gather call (a rows then b rows)
    percall = nidx // 16     # int16s per partition per call

    nc = bacc.Bacc(
        "TRN2",
        target_bir_lowering=False,
        debug=False,
        num_devices=ncores,
        num_swdge_queues=4,
    )
    tab = nc.dram_tensor("tab", [2 * N + 2, PW], mybir.dt.uint32, kind="ExternalInput")
    idxs = nc.dram_tensor(
        "idxs", [128, ncalls * percall], mybir.dt.int16, kind="ExternalInput"
    )
    outd = nc.dram_tensor("out", [GPC, PW], mybir.dt.uint32, kind="ExternalOutput")

    with tile.TileContext(nc) as tc, ExitStack() as ctx:
        pconst = ctx.enter_context(tc.tile_pool(name="const", bufs=1))
        pg = ctx.enter_context(tc.tile_pool(name="gather", bufs=4))
        po = ctx.enter_context(tc.tile_pool(name="osb", bufs=2))

        # Load indices with the gpsimd engine's own DMA so the gather's
        # dependency is engine-local (no slow cross-engine sem wait).
        idx_t = pconst.tile([128, idxs.shape[1]], mybir.dt.int16)
        nc.gpsimd.dma_start(idx_t[:], idxs[:])

        for h in range(ncalls):
            g_t = pg.tile([128, 2 * npc, PW], mybir.dt.uint32, tag="g")
            nc.gpsimd.dma_gather(
                g_t[:],
                tab[:],
                idx_t[:, h * percall : (h + 1) * percall],
                nidx,
                nidx,
                PW,
                single_packet=False,
                queue_num=h % 4,
            )
            for j in range(npc):
                bk = h * npc + j
                o_t = po.tile([128, PW], mybir.dt.uint32, tag=f"o{bk}")
                nc.vector.tensor_tensor(
                    o_t[:],
                    g_t[:, 2 * j, :],
                    g_t[:, 2 * j + 1, :],
                    op=_ALU[SCHED[bk]],
                )
                # Alternate the two HWDGE rings (sync=qSPDynamicHW,
                # scalar=qActDynamicHW) so output writes run in parallel.
                eng = nc.sync if bk % 2 == 0 else nc.scalar
                eng.dma_start(outd[bk * 128 : (bk + 1) * 128, :], o_t[:])
    nc.compile()
    return nc


# ---------------------------------------------------------------------------
# Host-side input prep
# ---------------------------------------------------------------------------


def _prep(x, gates, choices, ncalls=NCALLS):
    x8 = np.asarray(x, dtype=np.uint8)
    gates8 = np.asarray(gates, dtype=np.uint8)
    ch = np.asarray(choices, dtype=np.int64)

    # Packed doubled table (replicated on every core).
    xp = np.packbits(x8, axis=0)              # [B/8, N], bit MSB = lowest batch row
    tab = np.empty((2 * N + 2, PB), dtype=np.uint8)
    tab[:N] = xp.T
    tab[N : 2 * N] = ~tab[:N]
    tab[ROW_ONE] = 0xFF
    tab[ROW_ZERO] = 0x00
    tab32 = tab.view(np.uint32)

    # Bucket assignment: required-family gates first, flexible gates pad.
    tt = (gates8 << np.arange(4, dtype=np.uint8)).sum(axis=1).astype(np.int64)
    req = {op: [t for t in range(16) if _FAMS[t] == {op}] for op in _OPS}
    flex = [t for t in range(16) if len(_FAMS[t]) == 3]
    assert sum(len(v) for v in req.values()) + len(flex) == 16

    gid = np.arange(G)
    flex_pool = gid[np.isin(tt, flex)]
    fp = 0
    slots = {}
    for op in _OPS:
        need = gid[np.isin(tt, req[op])]
        pad = CAP[op] - len(need)
        assert pad >= 0, f"bucket {op} overflow: {len(need)} > {CAP[op]}"
        slots[op] = np.concatenate([need, flex_pool[fp : fp + pad]])
        fp += pad
    assert fp == len(flex_pool)

    # Device gate order (core-major, schedule-major) + operand row indices.
    npcg = {"and": 3 * 128, "or": 3 * 128, "xor": 2 * 128}
    perm = np.empty(G, dtype=np.int64)        # device row -> gate id
    ia = np.empty(G, dtype=np.int64)
    ib = np.empty(G, dtype=np.int64)
    r = 0
    for k in range(NCORES):
        for op in _OPS:
            g = slots[op][k * npcg[op] : (k + 1) * npcg[op]]
            lut = [_SEL[op][t] or (5, 5) for t in range(16)]  # (5,5) never used
            selA = np.array([s[0] for s in lut])[tt[g]]
            selB = np.array([s[1] for s in lut])[tt[g]]
            rows = np.stack(
                [ch[g, 0], ch[g, 0] + N, ch[g, 1], ch[g, 1] + N,
                 np.full(len(g), ROW_ONE), np.full(len(g), ROW_ZERO)]
            )
            n = len(g)
            perm[r : r + n] = g
            ia[r : r + n] = rows[selA, np.arange(n)]
            ib[r : r + n] = rows[selB, np.arange(n)]
            r += n
    assert r == G

    # Wrapped int16 index layout per core: ncalls calls; call h covers
    # npc gate blocks -> flat order [a rows of npc blocks, b rows of npc
    # blocks] interleaved as [a(blk0),b(blk0),a(blk1),b(blk1),...] to
    # match tile slots (2j, 2j+1).
    npc = NBLK // ncalls
    in_maps = []
    for k in range(NCORES):
        s = slice(k * GPC, (k + 1) * GPC)
        iak = ia[s].reshape(NBLK, 128)
        ibk = ib[s].reshape(NBLK, 128)
        cols = []
        for h in range(ncalls):
            inter = np.empty((2 * npc, 128), dtype=np.int16)
            inter[0::2] = iak[h * npc : (h + 1) * npc]
            inter[1::2] = ibk[h * npc : (h + 1) * npc]
            flat = inter.reshape(-1)
            wrapped = flat.reshape(-1, 16).T      # [16, nidx/16]
            cols.append(np.tile(wrapped, (8, 1)))  # [128, nidx/16]
        idxs_np = np.ascontiguousarray(np.concatenate(cols, axis=1))
        in_maps.append({"tab": tab32, "idxs": idxs_np})
    return in_maps, perm


# ---------------------------------------------------------------------------
# Entry point
# ---------------------------------------------------------------------------

_NC_CACHE = {}


def _get_nc():
    if "nc" not in _NC_CACHE:
        _NC_CACHE["nc"] = build_nc()
    return _NC_CACHE["nc"]


def kernel(x, gates, choices):
    in_maps, perm = _prep(x, gates, choices)
    nc = _get_nc()
    res = run_bass_kernel_spmd(nc, in_maps, list(range(NCORES)))
    packed = np.concatenate(
        [res.results[k]["out"].view(np.uint8) for k in range(NCORES)], axis=0
    )
    ordered = np.empty_like(packed)
    ordered[perm] = packed                    # un-permute gate rows
    up = np.unpackbits(ordered, axis=1)       # [G, B] 0/1 uint8
    return up.view(np.bool_).T                # [B, G] bool view
